# revision 14
# baseline (speedup 1.0000x reference)
"""Trainium2 Bass kernel for nn_EnvEncoder (7-branch MLP + 2x LayerNorm).

Contract: kernel(**inputs) takes the FULL unsharded inputs (x: [524288, 94] f32
plus small weights) and returns the FULL output [524288, 128] f32.

Strategy (pure data parallel over 8 cores, 65536 rows/core), v2:
  - Host folds the 7 branch Linears into one block-diagonal W1 [95, 160]
    (row 94 = bias row; x transposed + ones row appended on host).
    W2 = w_fuse row-centered (wc) + centered bias bc.
  - Algebra: relu(LN1(h)) = rstd1 * relu(h - mu1)  (rstd1 > 0, g1=1, b1=0).
    v = relu(h - mu1), with std1 = sqrt(var1+eps) carried as an extra column:
      p2 = v@wc + std1*bc   (std column multiplies the bc row of W2b)
      h2 = u@wc + bc = rstd1 * p2, mean_j(h2) = 0 exactly (wc, bc centered)
      var2 = rstd1^2 * mean_j(p2^2)
      out = relu(h2 * rstd2) = relu((rstd1*rstd2) * p2)
  - Device per 128-sample tile (samples on partitions), SG tiles per
    supergroup:
      mm1 (PE, 3 tiles/bank) -> ACT relu copy with accum_out (row-sum s1 free)
      sumsq s2 via fused TTR (DVE); grouped scalar math -> mu, rstd, std, rec
      v-affine: cols 0:128 -> slabA (DVE TS dual sub+max),
                cols 128:160 -> slabB cols 0:32 (GPSIMD TS), std -> slabB col 32
      2 batched XBAR DMA transposes per SG: slabA -> vTa [128, SG, 128],
        slabB -> vTb (rows 0:33 valid) — no PE transpose, no PSUM round trip
      mm2 (PE, 2 accumulating matmuls, K=128 + K=33)
      LN2 sumsq via TTR (DVE); final relu(rr*p2) (ACT, scale=rr)
  - Output bf16, partition-major DRAM layout [128, n_tiles, 128]; host
    reassembles and casts to f32.
"""

import os
import numpy as np
import ml_dtypes

import concourse.bass as bass
import concourse.bacc as bacc
import concourse.tile as tile
from concourse import mybir
from concourse.bass_utils import run_bass_kernel_spmd

B_TOTAL = 524288
N_CORES = 8
B_CORE = B_TOTAL // N_CORES  # 65536
P = 128                       # samples per tile (partition dim)
K1 = 95                       # 94 features + ones row
F1 = 160                      # hidden features
F2 = 128                      # output features
SG = 12                       # tiles per supergroup
G1 = 3                        # mm1 outputs per PSUM bank
G2 = 3                        # mm2 outputs per PSUM bank
KB = 33                       # mm2b contraction: 32 hidden feats + std col
EPS = 1e-5

_BRANCHES = [
    ("month", 0, 12, 0, 32),
    ("area", 12, 18, 32, 48),
    ("icls", 18, 24, 48, 64),
    ("scalar", 24, 26, 64, 80),
    ("long", 26, 62, 80, 112),
    ("lat", 62, 74, 112, 128),
    ("hist", 74, 94, 128, 160),
]

TRACE = False  # set by test harness for profiled runs

# Implementation choices (validated per-op on HW by micro_bisect rounds)
S2_IMPL = os.environ.get("ENVENC_S2", "act")      # act | stt | ttred
S2M_IMPL = os.environ.get("ENVENC_S2M", "act")    # act | stt
FINAL_ENG = os.environ.get("ENVENC_FINAL", "dve")  # dve | act
AFFB_ENG = os.environ.get("ENVENC_AFFB", "dve")   # dve | gp

_PROGRAM_CACHE = {}
LAST_RESULTS = None


def _iter_chunks(n, size):
    out = []
    i = 0
    while i < n:
        out.append((i, min(size, n - i)))
        i += size
    return out


def build_program(n_tiles):
    bf16 = mybir.dt.bfloat16
    f32 = mybir.dt.float32
    FRelu = mybir.ActivationFunctionType.Relu
    FSqrt = mybir.ActivationFunctionType.Sqrt
    FSquare = mybir.ActivationFunctionType.Square
    mult = mybir.AluOpType.mult
    add = mybir.AluOpType.add
    sub = mybir.AluOpType.subtract
    amax = mybir.AluOpType.max

    nc = bacc.Bacc("TRN2", target_bir_lowering=False, debug=False,
                   num_devices=N_CORES)

    n_rows = n_tiles * P
    xT = nc.dram_tensor("xT", [K1, n_rows], bf16, kind="ExternalInput").ap()
    w1 = nc.dram_tensor("w1", [K1, F1], bf16, kind="ExternalInput").ap()
    w2a = nc.dram_tensor("w2a", [P, F2], bf16, kind="ExternalInput").ap()
    # w2b rows 0:32 = wc[128:160], row 32 = bc
    w2b = nc.dram_tensor("w2b", [KB, F2], bf16, kind="ExternalInput").ap()
    out = nc.dram_tensor("out", [P, n_tiles, F2], bf16,
                         kind="ExternalOutput").ap()

    with tile.TileContext(nc) as tc:
        with (
            tc.tile_pool(name="consts", bufs=1) as cpool,
            tc.tile_pool(name="xc", bufs=2) as xpool,
            tc.tile_pool(name="psum1", bufs=3, space="PSUM") as p1pool,
            tc.tile_pool(name="hr", bufs=2) as hrpool,
            tc.tile_pool(name="slabA", bufs=2) as sApool,
            tc.tile_pool(name="slabB", bufs=2) as sBpool,
            tc.tile_pool(name="vTa", bufs=2) as vTapool,
            tc.tile_pool(name="vTb", bufs=2) as vTbpool,
            tc.tile_pool(name="stats", bufs=2) as stpool,
            tc.tile_pool(name="psum2", bufs=3, space="PSUM") as p2pool,
            tc.tile_pool(name="sq", bufs=4) as sqpool,
            tc.tile_pool(name="st2", bufs=2) as st2pool,
            tc.tile_pool(name="outb", bufs=2) as opool,
        ):
            # --- persistent constants ---
            w1_t = cpool.tile([K1, F1], bf16, tag="w1")
            nc.sync.dma_start(w1_t[:], w1)
            w2a_t = cpool.tile([P, F2], bf16, tag="w2a")
            nc.sync.dma_start(w2a_t[:], w2a)
            w2b_t = cpool.tile([KB, F2], bf16, tag="w2b")
            nc.sync.dma_start(w2b_t[:], w2b)

            for sg0, sg_n in _iter_chunks(n_tiles, SG):
                # --- load x chunk: [95, sg_n*128] ---
                xc = xpool.tile([K1, SG * P], bf16, tag="xc")
                nc.sync.dma_start(xc[:, 0:sg_n * P],
                                  xT[:, sg0 * P:(sg0 + sg_n) * P])

                # --- mm1; relu(+s1) per tile; sumsq s2 per tile ---
                hr = hrpool.tile([P, SG, F1], bf16, tag="hr")
                s1 = stpool.tile([P, SG], f32, tag="s1")
                s2 = stpool.tile([P, SG], f32, tag="s2")
                for g0, g_n in _iter_chunks(sg_n, G1):
                    p1 = p1pool.tile([P, G1 * F1], f32, tag="p1")
                    for i in range(g_n):
                        nc.tensor.matmul(
                            p1[:, i * F1:(i + 1) * F1],
                            lhsT=xc[:, (g0 + i) * P:(g0 + i + 1) * P],
                            rhs=w1_t[:],
                            start=True, stop=True,
                        )
                    for i in range(g_n):
                        t = g0 + i
                        nc.scalar.activation(hr[:, t, :],
                                             p1[:, i * F1:(i + 1) * F1],
                                             FRelu, accum_out=s1[:, t:t + 1])
                        if S2_IMPL == "stt":
                            # sq = (hr*1)*hr with accum -> s2 (one fused op)
                            sq = sqpool.tile([P, F1], bf16, tag="sq1")
                            nc.vector.scalar_tensor_tensor(
                                sq[:], hr[:, t, :], 1.0, hr[:, t, :],
                                mult, mult, accum_out=s2[:, t:t + 1])
                        elif S2_IMPL == "ttred":
                            sq = sqpool.tile([P, F1], bf16, tag="sq1")
                            nc.vector.tensor_tensor(sq[:], hr[:, t, :],
                                                    hr[:, t, :], mult)
                            nc.vector.tensor_reduce(
                                s2[:, t:t + 1], sq[:],
                                mybir.AxisListType.XYZW, add)
                        else:  # act
                            sq = sqpool.tile([P, F1], bf16, tag="sq1")
                            nc.scalar.activation(sq[:], hr[:, t, :],
                                                 FSquare,
                                                 accum_out=s2[:, t:t + 1])

                # --- grouped stat math: mu, veps, rec, rstd, std ---
                mu = stpool.tile([P, SG], f32, tag="mu")
                nc.vector.tensor_scalar(mu[:, 0:sg_n], s1[:, 0:sg_n],
                                        1.0 / F1, None, mult)
                t1 = stpool.tile([P, SG], f32, tag="t1")
                nc.vector.tensor_scalar(t1[:, 0:sg_n], s2[:, 0:sg_n],
                                        1.0 / F1, EPS, mult, op1=add)
                q = stpool.tile([P, SG], f32, tag="q")
                nc.vector.tensor_tensor(q[:, 0:sg_n], mu[:, 0:sg_n],
                                        mu[:, 0:sg_n], mult)
                veps = stpool.tile([P, SG], f32, tag="veps")
                nc.vector.tensor_tensor(veps[:, 0:sg_n], t1[:, 0:sg_n],
                                        q[:, 0:sg_n], sub)
                rec = stpool.tile([P, SG], f32, tag="rec")   # rstd^2
                nc.vector.reciprocal(rec[:, 0:sg_n], veps[:, 0:sg_n])
                rstd = stpool.tile([P, SG], f32, tag="rstd")
                nc.scalar.activation(rstd[:, 0:sg_n], rec[:, 0:sg_n], FSqrt)
                std = stpool.tile([P, SG], f32, tag="std")
                nc.scalar.activation(std[:, 0:sg_n], veps[:, 0:sg_n], FSqrt)

                # --- v-affine into contiguous slabs ---
                slabA = sApool.tile([P, SG, P], bf16, tag="slabA")
                slabB = sBpool.tile([P, SG, P], bf16, tag="slabB")
                eng_b = nc.gpsimd if AFFB_ENG == "gp" else nc.vector
                for t in range(sg_n):
                    nc.vector.tensor_scalar(
                        slabA[:, t, :], hr[:, t, 0:P],
                        mu[:, t:t + 1], 0.0, sub, op1=amax)
                    eng_b.tensor_scalar(
                        slabB[:, t, 0:32], hr[:, t, P:F1],
                        mu[:, t:t + 1], 0.0, sub, op1=amax)
                # std column (strided across tiles)
                eng_b.tensor_copy(slabB[:, 0:sg_n, 32], std[:, 0:sg_n])

                # --- batched XBAR transposes (SBUF->SBUF) ---
                vTa = vTapool.tile([P, SG, P], bf16, tag="vTa")
                nc.sync.dma_start_transpose(vTa[:, 0:sg_n, :],
                                            slabA[:, 0:sg_n, :])
                vTb = vTbpool.tile([P, SG, P], bf16, tag="vTb")
                nc.sync.dma_start_transpose(vTb[:, 0:sg_n, :],
                                            slabB[:, 0:sg_n, :])

                # --- mm2, LN2 sumsq, final ---
                outb = opool.tile([P, SG, F2], bf16, tag="outb")
                s2m = st2pool.tile([P, SG], f32, tag="s2m")
                pend = []

                def flush_final(pend, outb=outb):
                    if not pend:
                        return
                    lo = pend[0][1]
                    hi = pend[-1][1]
                    # veps2 = rec*s2m/128 + EPS; rr = rstd*sqrt(1/veps2)
                    ve2 = st2pool.tile([P, SG], f32, tag="ve2")
                    nc.vector.tensor_tensor(ve2[:, lo:hi + 1],
                                            s2m[:, lo:hi + 1],
                                            rec[:, lo:hi + 1], mult)
                    ve2b = st2pool.tile([P, SG], f32, tag="ve2b")
                    nc.vector.tensor_scalar(ve2b[:, lo:hi + 1],
                                            ve2[:, lo:hi + 1],
                                            1.0 / F2, EPS, mult, op1=add)
                    rec2 = st2pool.tile([P, SG], f32, tag="rec2")
                    nc.vector.reciprocal(rec2[:, lo:hi + 1],
                                         ve2b[:, lo:hi + 1])
                    rstd2 = st2pool.tile([P, SG], f32, tag="rstd2")
                    nc.scalar.activation(rstd2[:, lo:hi + 1],
                                         rec2[:, lo:hi + 1], FSqrt)
                    rr = st2pool.tile([P, SG], f32, tag="rr")
                    nc.vector.tensor_tensor(rr[:, lo:hi + 1],
                                            rstd[:, lo:hi + 1],
                                            rstd2[:, lo:hi + 1], mult)
                    for (p2t, t, slot) in pend:
                        psl = p2t[:, slot * F2:(slot + 1) * F2]
                        if FINAL_ENG == "dve":
                            nc.vector.tensor_scalar(
                                outb[:, t, :], psl, rr[:, t:t + 1],
                                0.0, mult, op1=amax)
                        else:
                            nc.scalar.activation(outb[:, t, :], psl, FRelu,
                                                 scale=rr[:, t:t + 1])

                p2t = None
                for t in range(sg_n):
                    slot = t % G2
                    if slot == 0:
                        p2t = p2pool.tile([P, G2 * F2], f32, tag="p2")
                    psl = p2t[:, slot * F2:(slot + 1) * F2]
                    nc.tensor.matmul(psl, lhsT=vTa[:, t, :], rhs=w2a_t[:],
                                     start=True, stop=False)
                    nc.tensor.matmul(psl, lhsT=vTb[0:KB, t, :],
                                     rhs=w2b_t[:],
                                     start=False, stop=True)
                    sq2 = sqpool.tile([P, F2], bf16, tag="sq2")
                    if S2M_IMPL == "stt":
                        nc.vector.scalar_tensor_tensor(
                            sq2[:], psl, 1.0, psl, mult, mult,
                            accum_out=s2m[:, t:t + 1])
                    else:  # act
                        nc.scalar.activation(sq2[:], psl, FSquare,
                                             accum_out=s2m[:, t:t + 1])
                    pend.append((p2t, t, slot))
                    if len(pend) == 6:
                        flush_final(pend)
                        pend = []
                flush_final(pend)

                # --- store supergroup output (partition-major layout) ---
                nc.sync.dma_start(out[:, sg0:sg0 + sg_n, :],
                                  outb[:, 0:sg_n, :])

    nc.compile()
    return nc


def _prep_host(inputs):
    bf = ml_dtypes.bfloat16
    x = np.asarray(inputs["x"], np.float32)
    assert x.shape == (B_TOTAL, 94), x.shape

    w1 = np.zeros((K1, F1), np.float32)
    for name, il, ih, ol, oh in _BRANCHES:
        w1[il:ih, ol:oh] = np.asarray(inputs[f"w_{name}"], np.float32)
        w1[94, ol:oh] = np.asarray(inputs[f"b_{name}"], np.float32)

    wf = np.asarray(inputs["w_fuse"], np.float32)
    bfu = np.asarray(inputs["b_fuse"], np.float32)
    wc = wf - wf.mean(axis=1, keepdims=True)
    bc = bfu - bfu.mean()
    w2a = np.ascontiguousarray(wc[0:P])               # [128, 128]
    w2b = np.concatenate([wc[P:F1], bc[None, :]], 0)  # [33, 128]

    xT = np.empty((K1, B_TOTAL), np.float32)
    xT[0:94] = x.T
    xT[94] = 1.0

    core_maps = []
    for c in range(N_CORES):
        core_maps.append({
            "xT": np.ascontiguousarray(
                xT[:, c * B_CORE:(c + 1) * B_CORE]).astype(bf),
            "w1": w1.astype(bf),
            "w2a": w2a.astype(bf),
            "w2b": w2b.astype(bf),
        })
    return core_maps


def _general_ln(inputs):
    ln1_g = np.asarray(inputs["ln1_g"], np.float32)
    ln1_b = np.asarray(inputs["ln1_b"], np.float32)
    ln2_g = np.asarray(inputs["ln2_g"], np.float32)
    ln2_b = np.asarray(inputs["ln2_b"], np.float32)
    return not (np.allclose(ln1_g, 1.0) and np.allclose(ln1_b, 0.0)
                and np.allclose(ln2_g, 1.0) and np.allclose(ln2_b, 0.0))


def kernel(**inputs):
    global LAST_RESULTS
    if _general_ln(inputs):
        import kernel_v1_backup as kv1
        kv1.TRACE = TRACE
        out = kv1.kernel(**inputs)
        LAST_RESULTS = kv1.LAST_RESULTS
        return out

    core_maps = _prep_host(inputs)
    n_tiles = B_CORE // P
    key = (n_tiles, S2_IMPL, S2M_IMPL, FINAL_ENG, AFFB_ENG)
    if key not in _PROGRAM_CACHE:
        _PROGRAM_CACHE[key] = build_program(n_tiles)
    nc = _PROGRAM_CACHE[key]

    res = run_bass_kernel_spmd(nc, core_maps, list(range(N_CORES)),
                               trace=TRACE)
    LAST_RESULTS = res
    out = np.empty((B_TOTAL, F2), np.float32)
    for c in range(N_CORES):
        o = res.results[c]["out"]  # [128, n_tiles, 128] bf16, partition-major
        out[c * B_CORE:(c + 1) * B_CORE] = (
            o.transpose(1, 0, 2).reshape(B_CORE, F2).astype(np.float32))
    return out
